# revision 52
# baseline (speedup 1.0000x reference)
"""Multi-head attention (N=4, L=2048, C=1024, H=16, D=64) on 8 TRN2 NeuronCores.

Sharding: core c -> batch n = c//2, head-group g = c%2 (8 heads each).
Each core computes its 8 heads' attention + the partial output projection
for batch n; the host sums the two partials per batch and adds the
constant bias term (b_out + b_v @ W_out).

v2: all projections (qkv in, V, out) run as fp8e4 DoubleRow matmuls
(256-deep contraction per pass, half the PE streaming); weights are
pre-scaled x64 on the host to stay clear of fp8 subnormals, and the
4096x score scale / 4096x output scale are folded into the exp scale
and the final y copy. Inputs ship as fp8 (half the DMA), y returns bf16.
reciprocal -> reciprocal_approx_fast (~5x cheaper on DVE).

Device-side layout (per core):
  xT   [C=1024, L=2048]  fp8e4 (x[n].T, host-transposed/cast)
  wqk  [C, 1024]         fp8e4 (64*W_in cols: 8 heads' q dims then k dims)
  wv   [C, 512]          fp8e4 (64*W_in v cols)
  wo   [512, F=1024]     fp8e4 (64*W_out rows for the 8 heads)
  bqk  [128, 8]          f32   (64x q/k bias, partition-major per j-tile)
  y    [L, F]            bf16  output partial (scaled back by 1/4096)

Pipeline (ACT exp is the roofline engine; everything else hides under it):
  - qT/kT = W^T @ xT (j on partitions), V = xT^T @ Wv (l on partitions)
  - scoresT[k, q] per head, row-tiled head pairs (K=64 -> rows 0-63 /
    64-127, hardware-concurrent)
  - exp on ACT (scale=1/(8*4096) fused, fp32 PSUM -> bf16 SBUF)
  - AV^T col-tiled pairs into separate PSUM banks; row sums as M=64
    ones-matmuls (replicated across 64 partitions) cross-placed into the
    sibling head's free bank rows -> reciprocal and normalize are fully
    partition-aligned, no broadcast needed
  - qkT projections for the next pair and the final y projection are
    interleaved into the attention chunks as PE filler work
"""

import sys
from contextlib import ExitStack

import numpy as np

sys.path.insert(0, "/opt/trn_rl_repo")

import ml_dtypes

import concourse.bass as bass
import concourse.tile as tile
from concourse import bacc, mybir
from concourse.bass_utils import run_bass_kernel_spmd

BF16 = mybir.dt.bfloat16
F32 = mybir.dt.float32
FP8 = mybir.dt.float8e4
DRMODE = mybir.MatmulPerfMode.DoubleRow
FT = mybir.ActivationFunctionType
MULT = mybir.AluOpType.mult

N, L, C, H, D = 4, 2048, 1024, 16, 64
QKV = H * D  # 1024
F = 1024  # output feature dim
HG = 8  # heads per core
NCORES = 8
WS = 32.0  # host q/k weight prescale (fp8 subnormal avoidance)
SCALE = float(D) ** -0.5 / (WS * WS)  # exp scale: 0.125 / 4096

CT = C // 128  # 8 c-tiles
CP = CT // 2  # 4 DoubleRow c-tile pairs
LT = L // 128  # 16 l-tiles
JQ = L // 512  # 4 q-chunks
KT = L // 128  # 16 k-tiles
NP = HG // 2  # 4 head pairs

# Globals for test harness introspection
TRACE = False
DEBUG = False
LAST_RESULTS = None


def _build_program() -> bass.Bass:
    nc = bacc.Bacc()

    xT_d = nc.declare_dram_parameter("xT", [C, L], BF16, isOutput=False)
    x8_d = nc.declare_dram_parameter("x8", [C, L], FP8, isOutput=False)
    wqk_d = nc.declare_dram_parameter("wqk", [C, 1024], FP8, isOutput=False)
    wv_d = nc.declare_dram_parameter("wv", [C, 512], BF16, isOutput=False)
    wo_d = nc.declare_dram_parameter("wo", [512, F], BF16, isOutput=False)
    bqk_d = nc.declare_dram_parameter("bqk", [128, 8], F32, isOutput=False)
    y_d = nc.declare_dram_parameter("y", [L, F], BF16, isOutput=True)
    if DEBUG:
        dbg_qkT_d = nc.declare_dram_parameter("dbg_qkT", [128, 8, 4, 512], BF16, isOutput=True)
        dbg_V_d = nc.declare_dram_parameter("dbg_V", [128, LT, 512], BF16, isOutput=True)
        dbg_outT_d = nc.declare_dram_parameter("dbg_outT", [128, NP, L], FP8, isOutput=True)

    with tile.TileContext(nc) as tc, ExitStack() as ctx:
        const_pool = ctx.enter_context(tc.tile_pool(name="const", bufs=1))
        qk_pool = ctx.enter_context(tc.tile_pool(name="qkT", bufs=1))
        v_pool = ctx.enter_context(tc.tile_pool(name="V", bufs=1))
        outT_pool = ctx.enter_context(tc.tile_pool(name="outT", bufs=1))
        exp_pool = ctx.enter_context(tc.tile_pool(name="expT", bufs=3))
        # one-shot half-chunk exp tile: chunk 3's first 8 score k-tiles run
        # in the prologue so ACT stays fed through chunk 0's V/AV block
        expX_pool = ctx.enter_context(tc.tile_pool(name="expX", bufs=1))
        r_pool = ctx.enter_context(tc.tile_pool(name="r", bufs=1))
        y_pool = ctx.enter_context(tc.tile_pool(name="y", bufs=2))
        wo_pool = ctx.enter_context(tc.tile_pool(name="wo", bufs=1))
        # PSUM: scores 2x2 banks + avA 1 + avB 1 + proj 2 = 8 banks
        ps_s = ctx.enter_context(tc.tile_pool(name="ps_s", bufs=2, space="PSUM"))
        ps_avA = ctx.enter_context(tc.tile_pool(name="ps_avA", bufs=1, space="PSUM"))
        ps_avB = ctx.enter_context(tc.tile_pool(name="ps_avB", bufs=1, space="PSUM"))
        ps_proj = ctx.enter_context(tc.tile_pool(name="ps_proj", bufs=1, space="PSUM"))

        ones64 = const_pool.tile([128, 64], BF16)
        nc.vector.memset(ones64[:], 1.0)
        bqk_sb = const_pool.tile([128, 8], F32)
        nc.sync.dma_start(bqk_sb[:], bqk_d[:])

        # qT/kT: [128, jt(8), jl(4), 512] ; jt 0-3 q dims, 4-7 k dims.
        # fp8: the score matmuls take fp8 operands in normal mode at full
        # speed; q/k noise only perturbs logits by ~0.006.
        qkT_sb = qk_pool.tile([128, 8, 4, 512], FP8)
        # V: [128, lt(16), 512]
        V_sb = v_pool.tile([128, LT, 512], BF16)
        # outT: [128, pair(4), L] (partitions = 2 heads x 64 dims)
        outT_sb = outT_pool.tile([128, NP, L], BF16)

        def qkT_proj_unit(xT_sb, wqk_sb, jt, lh):
            """qkT[j, l] = sum_c wqk[c, j] xT[c, l] for one (j-tile, L-half),
            as 4x2 DoubleRow matmuls (c-tile pairs)."""
            ps = ps_proj.tile([128, 2, 512], F32, tag="proj")
            for cp in range(CP):
                for lc in range(2):
                    nc.tensor.matmul(
                        ps[:, lc],
                        lhsT=wqk_sb[:, 2 * cp : 2 * cp + 2, jt * 128 : (jt + 1) * 128],
                        rhs=xT_sb[
                            :,
                            2 * cp : 2 * cp + 2,
                            lh * 1024 + lc * 512 : lh * 1024 + (lc + 1) * 512,
                        ],
                        start=(cp == 0),
                        stop=(cp == CP - 1),
                        perf_mode=DRMODE,
                    )
            nc.vector.tensor_scalar_add(
                qkT_sb[:, jt, 2 * lh : 2 * lh + 2, :], ps[:], bqk_sb[:, jt : jt + 1]
            )

        def score_kt(p, jq, expT, kt):
            """One k-tile of scoresT + its exp for head pair p, chunk jq."""
            S = ps_s.tile([128, 2, 512], F32, tag="s")
            jl, off = kt // 4, (kt % 4) * 128
            nc.tensor.matmul(
                S[:, 0],
                lhsT=qkT_sb[0:64, 4 + p, jl, off : off + 128],
                rhs=qkT_sb[0:64, p, jq, :],
                start=True,
                stop=True,
            )
            nc.tensor.matmul(
                S[:, 1],
                lhsT=qkT_sb[64:128, 4 + p, jl, off : off + 128],
                rhs=qkT_sb[64:128, p, jq, :],
                start=True,
                stop=True,
            )
            nc.scalar.activation(expT[:, kt], S[:], FT.Exp, scale=SCALE)

        def av_alloc():
            avA = ps_avA.tile([128, 512], F32, tag="avA")
            avB = ps_avB.tile([128, 512], F32, tag="avB")
            return avA, avB

        def av_mms(avA, avB, p, jq, expT, kts):
            """AV accumulation-group matmuls for the given k-tiles: both
            heads' AV into the avA bank (rows 0:64 / 64:128, col-tiled)."""
            hA, hB = 2 * p, 2 * p + 1
            for kt in kts:
                st, sp = kt == 0, kt == KT - 1
                nc.tensor.matmul(
                    avA[0:64],
                    lhsT=V_sb[:, kt, hA * 64 : hA * 64 + 64],
                    rhs=expT[:, kt, 0],
                    start=st,
                    stop=sp,
                )
                nc.tensor.matmul(
                    avA[64:128],
                    lhsT=V_sb[:, kt, hB * 64 : hB * 64 + 64],
                    rhs=expT[:, kt, 1],
                    start=st,
                    stop=sp,
                )

        def sum_mms(avA, avB, expT, kts):
            """Row sums, replicated across 64 partitions (M=64 ones), both
            heads into the avB bank (rows 0:64 / 64:128) so reciprocal and
            normalize run as single full-128-partition ops at base 0."""
            for kt in kts:
                st, sp = kt == 0, kt == KT - 1
                nc.tensor.matmul(
                    avB[0:64], lhsT=ones64[:], rhs=expT[:, kt, 0], start=st, stop=sp
                )
                nc.tensor.matmul(
                    avB[64:128], lhsT=ones64[:], rhs=expT[:, kt, 1], start=st, stop=sp
                )

        def norm_part(p, jq, avA, avB):
            # avA holds both heads' AV, avB both heads' sums. Reciprocal
            # straight off the sums bank, then one fused normalize multiply
            # (PSUM x SBUF) into bf16 outT; both banks release right after.
            r_sb = r_pool.tile([128, 512], F32, tag="r")
            nc.vector.reciprocal_approx_fast(r_sb[:], avB[:])
            cols = slice(jq * 512, (jq + 1) * 512)
            nc.vector.tensor_tensor(
                outT_sb[:, p, cols], avA[:], r_sb[:], MULT
            )

        def y_unit_slices(lt, pool=None, tag="proj"):
            """y[l, f] = sum_d outT[d, l] wo[d, f] for one l-tile: 4 matmuls
            at full N=1024 (accumulating over head pairs), split into two
            drippable slices of 2 matmuls; cast + DMA ride on the second."""
            box = {}

            def emit(fc, lt=lt):
                if fc == 0:
                    box["psy"] = (pool or ps_proj).tile(
                        [128, 2, 512], F32, tag=tag, name=f"psy_{lt}"
                    )
                    box["y"] = y_pool.tile([128, 1024], BF16, tag="y", name=f"y_{lt}")
                psy, y_sb = box["psy"], box["y"]
                for p in range(NP):
                    nc.tensor.matmul(
                        psy[:, fc],
                        lhsT=outT_sb[:, p, lt * 128 : (lt + 1) * 128],
                        rhs=wo_sb[:, p, fc * 512 : (fc + 1) * 512],
                        start=(p == 0),
                        stop=(p == NP - 1),
                    )
                nc.vector.tensor_copy(y_sb[:, fc * 512 : (fc + 1) * 512], psy[:, fc])
                if fc == 1:
                    # gpsimd queue: keeps output DMAs off the input-DMA queue
                    nc.gpsimd.dma_start(y_d[lt * 128 : (lt + 1) * 128, :], y_sb[:])

            return [lambda fc=fc: emit(fc) for fc in range(2)]

        with tc.tile_pool(name="xw", bufs=1) as xw_pool:
            # DMA order = critical-path order: wqk + x8 (gate the first q/k
            # projections and scores), then bf16 x + wv (gate only the V
            # projection, which runs later).
            x8_sb = xw_pool.tile([128, CT, L], FP8)
            wqk_sb = xw_pool.tile([128, CT, 1024], FP8)
            xT_r = xT_d.rearrange("(t p) l -> p t l", p=128)
            x8_r = x8_d.rearrange("(t p) l -> p t l", p=128)
            wqk_r = wqk_d.rearrange("(t p) j -> p t j", p=128)
            for ct in range(CT):
                nc.sync.dma_start(wqk_sb[:, ct], wqk_r[:, ct])
                nc.sync.dma_start(x8_sb[:, ct], x8_r[:, ct])

            def V_pair(xbf_pool, wv_sb, g, proj_pool=False):
                """V projection for l-tiles 2g, 2g+1 as 4 drippable slices of
                4 matmuls. The bf16 x for these output rows arrives as a
                small [128, CT, 256] DMA slice fetched per pair (no resident
                32KB bf16-x tile)."""
                xv = xbf_pool.tile([128, CT, 256], BF16, tag="xv", name=f"xv_{g}")
                nc.sync.dma_start(xv[:], xT_r[:, :, g * 256 : (g + 1) * 256])
                slices = []
                for i in range(2):
                    lt = 2 * g + i
                    box = {}

                    def emit(half, i=i, lt=lt, xv=xv, box=box):
                        if half == 0:
                            if proj_pool:
                                pool, tag = ps_proj, "proj"
                            else:
                                pool, tag = (
                                    (ps_avA, "avA") if lt % 2 == 0 else (ps_avB, "avB")
                                )
                            box["psv"] = pool.tile(
                                [128, 512], F32, tag=tag, name=f"psv_{lt}"
                            )
                        psv = box["psv"]
                        for ct in range(4 * half, 4 * half + 4):
                            nc.tensor.matmul(
                                psv[:],
                                lhsT=xv[:, ct, i * 128 : (i + 1) * 128],
                                rhs=wv_sb[:, ct, :],
                                start=(ct == 0),
                                stop=(ct == CT - 1),
                            )
                        if half == 1:
                            nc.vector.tensor_copy(V_sb[:, lt, :], psv[:])

                    slices.append(lambda e=emit: e(0))
                    slices.append(lambda e=emit: e(1))
                return slices

            def qkT_unit_slices(jt, lh):
                """A qkT projection unit split into 4 drippable slices of
                2 DoubleRow matmuls (the psum group spans the slices)."""
                box = {}

                def emit(i, jt=jt, lh=lh):
                    if i == 0:
                        box["ps"] = ps_proj.tile(
                            [128, 2, 512], F32, tag="proj", name=f"proj_{jt}_{lh}"
                        )
                    ps = box["ps"]
                    for lc in range(2):
                        nc.tensor.matmul(
                            ps[:, lc],
                            lhsT=wqk_sb[
                                :, 2 * i : 2 * i + 2, jt * 128 : (jt + 1) * 128
                            ],
                            rhs=x8_sb[
                                :,
                                2 * i : 2 * i + 2,
                                lh * 1024 + lc * 512 : lh * 1024 + (lc + 1) * 512,
                            ],
                            start=(i == 0),
                            stop=(i == CP - 1),
                            perf_mode=DRMODE,
                        )
                    if i == 3:
                        nc.vector.tensor_scalar_add(
                            qkT_sb[:, jt, 2 * lh : 2 * lh + 2, :],
                            ps[:],
                            bqk_sb[:, jt : jt + 1],
                        )

                return [lambda i=i: emit(i) for i in range(4)]

            # Emission schedule: per chunk c we emit its AV groups (paced by
            # its exps), then the first 4 score k-tiles of chunk c+1 woven
            # between the two halves of c's row-sum pass (the sums can only
            # start once the AV groups close, i.e. after c's last exp), then
            # the normalize, then the remaining score k-tiles of c+1 with
            # projection work dripped one slice per k-tile.
            with tc.tile_pool(name="wv", bufs=1) as wv_pool, tc.tile_pool(
                name="xbf", bufs=2
            ) as xbf_pool:
                wv_sb = wv_pool.tile([128, CT, 512], BF16)
                nc.sync.dma_start(wv_sb[:], wv_d.rearrange("(t p) j -> p t j", p=128))

                # jq-major chunk order: chunk ci = (pair ci%4, jq ci//4).
                # Each q-block's outT completes after its 4th pair chunk, so
                # the y projection spreads over the whole kernel instead of
                # piling into the tail. Scores run two chunks ahead (3 exp
                # bufs); dripped qkT units are scheduled so each completes
                # before the woven scores that read it.
                chunks = [(i % NP, i // NP) for i in range(NP * JQ)]
                qkT_proj_unit(x8_sb, wqk_sb, 4, 0)
                qkT_proj_unit(x8_sb, wqk_sb, 0, 0)
                exp0 = exp_pool.tile([128, KT, 2, 512], BF16, tag="expT")
                for kt in range(8):
                    score_kt(0, 0, exp0, kt)
                qkT_proj_unit(x8_sb, wqk_sb, 4, 1)
                d = qkT_unit_slices(5, 0) + qkT_unit_slices(1, 0)
                for kt in range(8, KT):
                    d.pop(0)()
                    score_kt(0, 0, exp0, kt)
                # V slices drip into the ACT-paced score slots (the PE is
                # otherwise idle ~50% of each exp-paced score step)
                exp1 = exp_pool.tile([128, KT, 2, 512], BF16, tag="expT")
                d = qkT_unit_slices(5, 1) + V_pair(xbf_pool, wv_sb, 0)
                for kt in range(8):
                    d.pop(0)()
                    score_kt(1, 0, exp1, kt)
                d = qkT_unit_slices(6, 0) + qkT_unit_slices(2, 0)
                for kt in range(8, KT):
                    d.pop(0)()
                    score_kt(1, 0, exp1, kt)
                exp2 = exp_pool.tile([128, KT, 2, 512], BF16, tag="expT")
                d = qkT_unit_slices(6, 1) + V_pair(xbf_pool, wv_sb, 1)
                for kt in range(8):
                    d.pop(0)()
                    score_kt(2, 0, exp2, kt)
                d = qkT_unit_slices(7, 0) + qkT_unit_slices(3, 0)
                for kt in range(8, KT):
                    d.pop(0)()
                    score_kt(2, 0, exp2, kt)
                # early half of chunk (3,0)'s scores into the one-shot expX
                # tile (fed by the pair-3 q/k units just dripped above),
                # with the remaining prologue V pairs interleaved
                expX = expX_pool.tile([128, 4, 2, 512], BF16, tag="expX")
                d = V_pair(xbf_pool, wv_sb, 2) + V_pair(xbf_pool, wv_sb, 3)
                for kt in range(4):
                    d.pop(0)()
                    d.pop(0)()
                    score_kt(3, 0, expX, kt)

                exps = {0: exp0, 1: exp1, 2: exp2}
                # chunk (0,0) inline: V pairs 4-7 woven into its AV phase
                p, jq = chunks[0]
                expT = exps.pop(0)
                avA, avB = av_alloc()
                # ps_avA/avB hold this chunk's accumulators; route the V
                # psums through the proj pool instead
                vq = []
                for g in range(4, 8):
                    vq.extend(V_pair(xbf_pool, wv_sb, g, proj_pool=True))
                for kt in range(KT):
                    if vq:
                        vq.pop(0)()
                    av_mms(avA, avB, p, jq, expT, [kt])
                sum_mms(avA, avB, expT, range(0, KT))
                norm_part(p, jq, avA, avB)

            # wo loads after the prologue peak (first y unit at chunk 4),
            # reusing the SBUF freed by the wv/xbf pools
            wo_sb = wo_pool.tile([128, 4, F], BF16)
            nc.sync.dma_start(wo_sb[:], wo_d.rearrange("(t p) f -> p t f", p=128))

            # remaining qkT units (k1 for pair 3, then the lh=1 q units) and
            # the first 12 y units, dripped into specific chunks:
            #  - (7,1) at ci=1 completes before the (3,0) weave needs it (kt8)
            #  - q1 units (p,1) at ci 2-5, each well before the (p,2) weave
            #  - y l-tile lt drips once outT[:, :, jq(lt)] is complete, i.e.
            #    after chunk (3, jq) = ci 4*jq+3
            drip_sched = {
                2: [("qk", 0, 1)],
                3: [("qk", 1, 1)],
                4: [("qk", 2, 1), ("y", 0)],
                5: [("qk", 3, 1), ("y", 1)],
                6: [("y", 2)],
                7: [("y", 3)],
                8: [("y", 4)],
                9: [("y", 5)],
                10: [("y", 6)],
                11: [("y", 7)],
                12: [("y", 8)],
                13: [("y", 9)],
                14: [("y", 10)],
                15: [("y", 11)],
            }
            for ci in range(1, len(chunks)):
                p, jq = chunks[ci]
                nxt = chunks[ci + 2] if ci + 2 < len(chunks) else None
                emit_nxt = nxt is not None and (ci + 2) not in exps
                if emit_nxt:
                    exps[ci + 2] = exp_pool.tile([128, KT, 2, 512], BF16, tag="expT", name=f"expT_{ci+2}")
                expT = exps.pop(ci)
                avA, avB = av_alloc()
                # chunk 3's kt0-7 scores already ran in the prologue (expX),
                # so ci=1 only weaves its kt8-15
                wq = list(range(4, KT)) if ci == 1 else list(range(KT))

                def w():
                    if emit_nxt and wq:
                        score_kt(*nxt, exps[ci + 2], wq.pop(0))

                # the pair-3 k1 unit rides the ci=1 av phase (complete
                # before the first kt8 weave slot needs it)
                avdrip = qkT_unit_slices(7, 1) if ci == 1 else []
                for kt in range(KT - 1):
                    if avdrip:
                        avdrip.pop(0)()
                    if ci == 3:
                        av_mms(avA, avB, p, jq, expX if kt < 4 else expT, [kt])
                    else:
                        av_mms(avA, avB, p, jq, expT, [kt])
                    if kt == 9:
                        # mid-av weave slot: the S pool has drained the
                        # previous chunk's last exps by now, so this feeds
                        # ACT ~2us earlier at each chunk boundary without
                        # risking a stall at the head of the PE queue
                        w()
                w()
                av_mms(avA, avB, p, jq, expT, [KT - 1])
                w()
                if ci == 3:
                    sum_mms(avA, avB, expX, range(0, 4))
                    sum_mms(avA, avB, expT, range(4, 6))
                else:
                    sum_mms(avA, avB, expT, range(0, 6))
                w()
                sum_mms(avA, avB, expT, range(6, 11))
                w()
                sum_mms(avA, avB, expT, range(11, KT))
                w()
                norm_part(p, jq, avA, avB)

                drip = []
                for job in drip_sched.get(ci, []):
                    if job[0] == "qk":
                        drip.extend(qkT_unit_slices(job[1], job[2]))
                    else:
                        drip.extend(y_unit_slices(job[1]))
                for _ in range(5, KT):
                    if drip:
                        drip.pop(0)()
                    w()
                while drip:
                    drip.pop(0)()
                while emit_nxt and wq:
                    w()

            # tail: the last q-block's y units, double-buffered through the
            # now-idle score psum pool
            for lt in range(12, 16):
                for s in y_unit_slices(lt, pool=ps_s, tag="s"):
                    s()

            if DEBUG:
                nc.gpsimd.dma_start(dbg_qkT_d[:], qkT_sb[:])
                nc.gpsimd.dma_start(dbg_V_d[:], V_sb[:])
                nc.gpsimd.dma_start(dbg_outT_d[:], outT_sb[:])

    nc.finalize()
    return nc


_NC_CACHE = None


def _get_program():
    global _NC_CACHE
    if _NC_CACHE is None:
        _NC_CACHE = _build_program()
    return _NC_CACHE


def _make_in_maps(x, W_in, b_in, W_out):
    f8 = ml_dtypes.float8_e4m3
    bf = ml_dtypes.bfloat16
    in_maps = []
    for c in range(NCORES):
        n, g = c // 2, c % 2
        h0 = g * HG  # first global head
        j0 = h0 * D  # 512*g
        xT = np.ascontiguousarray(x[n].T).astype(bf)  # [C, L]
        x8 = xT.astype(f8)
        wqk = (
            WS
            * np.concatenate(
                [W_in[:, j0 : j0 + 512], W_in[:, QKV + j0 : QKV + j0 + 512]], axis=1
            )
        ).astype(f8)
        wv = np.ascontiguousarray(W_in[:, 2 * QKV + j0 : 2 * QKV + j0 + 512]).astype(bf)
        wo = np.ascontiguousarray(W_out[j0 : j0 + 512, :]).astype(bf)
        bqk = (
            (WS * np.concatenate([b_in[j0 : j0 + 512], b_in[QKV + j0 : QKV + j0 + 512]]))
            .astype(np.float32)
            .reshape(8, 128)
            .T.copy()
        )
        in_maps.append(
            {"xT": xT, "x8": x8, "wqk": wqk, "wv": wv, "wo": wo, "bqk": bqk}
        )
    return in_maps


def kernel(x, W_in, b_in, W_out, b_out):
    global LAST_RESULTS
    x = np.asarray(x, dtype=np.float32)
    W_in = np.asarray(W_in, dtype=np.float32)
    b_in = np.asarray(b_in, dtype=np.float32)
    W_out = np.asarray(W_out, dtype=np.float32)
    b_out = np.asarray(b_out, dtype=np.float32)

    nc = _get_program()
    in_maps = _make_in_maps(x, W_in, b_in, W_out)
    res = run_bass_kernel_spmd(nc, in_maps, list(range(NCORES)), trace=TRACE)
    LAST_RESULTS = res

    # host bias: b_out + b_v @ W_out  (b_v enters linearly through the
    # softmax-normalized value average: A@(V+b_v) = A@V + b_v)
    host_bias = (
        b_out.astype(np.float64)
        + b_in[2 * QKV :].astype(np.float64) @ W_out.astype(np.float64)
    ).astype(np.float32)

    out = np.empty((N, L, F), dtype=np.float32)
    for n in range(N):
        y0 = np.asarray(res.results[2 * n]["y"], dtype=np.float32)
        y1 = np.asarray(res.results[2 * n + 1]["y"], dtype=np.float32)
        out[n] = y0 + y1 + host_bias
    return out


# revision 53
# speedup vs baseline: 1.0089x; 1.0089x over previous
"""Multi-head attention (N=4, L=2048, C=1024, H=16, D=64) on 8 TRN2 NeuronCores.

Sharding: core c -> batch n = c//2, head-group g = c%2 (8 heads each).
Each core computes its 8 heads' attention + the partial output projection
for batch n; the host sums the two partials per batch and adds the
constant bias term (b_out + b_v @ W_out).

v2: all projections (qkv in, V, out) run as fp8e4 DoubleRow matmuls
(256-deep contraction per pass, half the PE streaming); weights are
pre-scaled x64 on the host to stay clear of fp8 subnormals, and the
4096x score scale / 4096x output scale are folded into the exp scale
and the final y copy. Inputs ship as fp8 (half the DMA), y returns bf16.
reciprocal -> reciprocal_approx_fast (~5x cheaper on DVE).

Device-side layout (per core):
  xT   [C=1024, L=2048]  fp8e4 (x[n].T, host-transposed/cast)
  wqk  [C, 1024]         fp8e4 (64*W_in cols: 8 heads' q dims then k dims)
  wv   [C, 512]          fp8e4 (64*W_in v cols)
  wo   [512, F=1024]     fp8e4 (64*W_out rows for the 8 heads)
  bqk  [128, 8]          f32   (64x q/k bias, partition-major per j-tile)
  y    [L, F]            bf16  output partial (scaled back by 1/4096)

Pipeline (ACT exp is the roofline engine; everything else hides under it):
  - qT/kT = W^T @ xT (j on partitions), V = xT^T @ Wv (l on partitions)
  - scoresT[k, q] per head, row-tiled head pairs (K=64 -> rows 0-63 /
    64-127, hardware-concurrent)
  - exp on ACT (scale=1/(8*4096) fused, fp32 PSUM -> bf16 SBUF)
  - AV^T col-tiled pairs into separate PSUM banks; row sums as M=64
    ones-matmuls (replicated across 64 partitions) cross-placed into the
    sibling head's free bank rows -> reciprocal and normalize are fully
    partition-aligned, no broadcast needed
  - qkT projections for the next pair and the final y projection are
    interleaved into the attention chunks as PE filler work
"""

import sys
from contextlib import ExitStack

import numpy as np

sys.path.insert(0, "/opt/trn_rl_repo")

import ml_dtypes

import concourse.bass as bass
import concourse.tile as tile
from concourse import bacc, mybir
from concourse.bass_utils import run_bass_kernel_spmd

BF16 = mybir.dt.bfloat16
F32 = mybir.dt.float32
FP8 = mybir.dt.float8e4
DRMODE = mybir.MatmulPerfMode.DoubleRow
FT = mybir.ActivationFunctionType
MULT = mybir.AluOpType.mult

N, L, C, H, D = 4, 2048, 1024, 16, 64
QKV = H * D  # 1024
F = 1024  # output feature dim
HG = 8  # heads per core
NCORES = 8
WS = 32.0  # host q/k weight prescale (fp8 subnormal avoidance)
SCALE = float(D) ** -0.5 / (WS * WS)  # exp scale: 0.125 / 4096

CT = C // 128  # 8 c-tiles
CP = CT // 2  # 4 DoubleRow c-tile pairs
LT = L // 128  # 16 l-tiles
JQ = L // 512  # 4 q-chunks
KT = L // 128  # 16 k-tiles
NP = HG // 2  # 4 head pairs

# Globals for test harness introspection
TRACE = False
DEBUG = False
LAST_RESULTS = None


def _build_program() -> bass.Bass:
    nc = bacc.Bacc()

    xT_d = nc.declare_dram_parameter("xT", [C, L], BF16, isOutput=False)
    x8_d = nc.declare_dram_parameter("x8", [C, L], FP8, isOutput=False)
    wqk_d = nc.declare_dram_parameter("wqk", [C, 1024], FP8, isOutput=False)
    wv_d = nc.declare_dram_parameter("wv", [C, 512], BF16, isOutput=False)
    wo_d = nc.declare_dram_parameter("wo", [512, F], BF16, isOutput=False)
    bqk_d = nc.declare_dram_parameter("bqk", [128, 8], F32, isOutput=False)
    y_d = nc.declare_dram_parameter("y", [L, F], BF16, isOutput=True)
    if DEBUG:
        dbg_qkT_d = nc.declare_dram_parameter("dbg_qkT", [128, 8, 4, 512], BF16, isOutput=True)
        dbg_V_d = nc.declare_dram_parameter("dbg_V", [128, LT, 512], BF16, isOutput=True)
        dbg_outT_d = nc.declare_dram_parameter("dbg_outT", [128, NP, L], FP8, isOutput=True)

    with tile.TileContext(nc) as tc, ExitStack() as ctx:
        const_pool = ctx.enter_context(tc.tile_pool(name="const", bufs=1))
        qk_pool = ctx.enter_context(tc.tile_pool(name="qkT", bufs=1))
        v_pool = ctx.enter_context(tc.tile_pool(name="V", bufs=1))
        outT_pool = ctx.enter_context(tc.tile_pool(name="outT", bufs=1))
        exp_pool = ctx.enter_context(tc.tile_pool(name="expT", bufs=3))
        # one-shot half-chunk exp tile: chunk 3's first 8 score k-tiles run
        # in the prologue so ACT stays fed through chunk 0's V/AV block
        expX_pool = ctx.enter_context(tc.tile_pool(name="expX", bufs=1))
        r_pool = ctx.enter_context(tc.tile_pool(name="r", bufs=1))
        y_pool = ctx.enter_context(tc.tile_pool(name="y", bufs=2))
        wo_pool = ctx.enter_context(tc.tile_pool(name="wo", bufs=1))
        # PSUM: scores 2x2 banks + avA 1 + avB 1 + proj 2 = 8 banks
        ps_s = ctx.enter_context(tc.tile_pool(name="ps_s", bufs=2, space="PSUM"))
        ps_avA = ctx.enter_context(tc.tile_pool(name="ps_avA", bufs=1, space="PSUM"))
        ps_avB = ctx.enter_context(tc.tile_pool(name="ps_avB", bufs=1, space="PSUM"))
        ps_proj = ctx.enter_context(tc.tile_pool(name="ps_proj", bufs=1, space="PSUM"))

        ones64 = const_pool.tile([128, 64], BF16)
        nc.vector.memset(ones64[:], 1.0)
        bqk_sb = const_pool.tile([128, 8], F32)
        nc.sync.dma_start(bqk_sb[:], bqk_d[:])

        # qT/kT: [128, jt(8), jl(4), 512] ; jt 0-3 q dims, 4-7 k dims.
        # fp8: the score matmuls take fp8 operands in normal mode at full
        # speed; q/k noise only perturbs logits by ~0.006.
        qkT_sb = qk_pool.tile([128, 8, 4, 512], FP8)
        # V: [128, lt(16), 512]
        V_sb = v_pool.tile([128, LT, 512], BF16)
        # outT: [128, pair(4), L] (partitions = 2 heads x 64 dims)
        outT_sb = outT_pool.tile([128, NP, L], BF16)

        def qkT_proj_unit(xT_sb, wqk_sb, jt, lh):
            """qkT[j, l] = sum_c wqk[c, j] xT[c, l] for one (j-tile, L-half),
            as 4x2 DoubleRow matmuls (c-tile pairs)."""
            ps = ps_proj.tile([128, 2, 512], F32, tag="proj")
            for cp in range(CP):
                for lc in range(2):
                    nc.tensor.matmul(
                        ps[:, lc],
                        lhsT=wqk_sb[:, 2 * cp : 2 * cp + 2, jt * 128 : (jt + 1) * 128],
                        rhs=xT_sb[
                            :,
                            2 * cp : 2 * cp + 2,
                            lh * 1024 + lc * 512 : lh * 1024 + (lc + 1) * 512,
                        ],
                        start=(cp == 0),
                        stop=(cp == CP - 1),
                        perf_mode=DRMODE,
                    )
            nc.vector.tensor_scalar_add(
                qkT_sb[:, jt, 2 * lh : 2 * lh + 2, :], ps[:], bqk_sb[:, jt : jt + 1]
            )

        def score_kt(p, jq, expT, kt):
            """One k-tile of scoresT + its exp for head pair p, chunk jq."""
            S = ps_s.tile([128, 2, 512], F32, tag="s")
            jl, off = kt // 4, (kt % 4) * 128
            nc.tensor.matmul(
                S[:, 0],
                lhsT=qkT_sb[0:64, 4 + p, jl, off : off + 128],
                rhs=qkT_sb[0:64, p, jq, :],
                start=True,
                stop=True,
            )
            nc.tensor.matmul(
                S[:, 1],
                lhsT=qkT_sb[64:128, 4 + p, jl, off : off + 128],
                rhs=qkT_sb[64:128, p, jq, :],
                start=True,
                stop=True,
            )
            nc.scalar.activation(expT[:, kt], S[:], FT.Exp, scale=SCALE)

        def av_alloc():
            avA = ps_avA.tile([128, 512], F32, tag="avA")
            avB = ps_avB.tile([128, 512], F32, tag="avB")
            return avA, avB

        def av_mms(avA, avB, p, jq, expT, kts):
            """AV accumulation-group matmuls for the given k-tiles: both
            heads' AV into the avA bank (rows 0:64 / 64:128, col-tiled)."""
            hA, hB = 2 * p, 2 * p + 1
            for kt in kts:
                st, sp = kt == 0, kt == KT - 1
                nc.tensor.matmul(
                    avA[0:64],
                    lhsT=V_sb[:, kt, hA * 64 : hA * 64 + 64],
                    rhs=expT[:, kt, 0],
                    start=st,
                    stop=sp,
                )
                nc.tensor.matmul(
                    avA[64:128],
                    lhsT=V_sb[:, kt, hB * 64 : hB * 64 + 64],
                    rhs=expT[:, kt, 1],
                    start=st,
                    stop=sp,
                )

        def sum_mms(avA, avB, expT, kts):
            """Row sums, replicated across 64 partitions (M=64 ones), both
            heads into the avB bank (rows 0:64 / 64:128) so reciprocal and
            normalize run as single full-128-partition ops at base 0."""
            for kt in kts:
                st, sp = kt == 0, kt == KT - 1
                nc.tensor.matmul(
                    avB[0:64], lhsT=ones64[:], rhs=expT[:, kt, 0], start=st, stop=sp
                )
                nc.tensor.matmul(
                    avB[64:128], lhsT=ones64[:], rhs=expT[:, kt, 1], start=st, stop=sp
                )

        def norm_part(p, jq, avA, avB):
            # avA holds both heads' AV, avB both heads' sums. Reciprocal
            # straight off the sums bank, then one fused normalize multiply
            # (PSUM x SBUF) into bf16 outT; both banks release right after.
            r_sb = r_pool.tile([128, 512], F32, tag="r")
            nc.vector.reciprocal_approx_fast(r_sb[:], avB[:])
            cols = slice(jq * 512, (jq + 1) * 512)
            nc.vector.tensor_tensor(
                outT_sb[:, p, cols], avA[:], r_sb[:], MULT
            )

        def y_unit_slices(lt, pool=None, tag="proj"):
            """y[l, f] = sum_d outT[d, l] wo[d, f] for one l-tile: 4 matmuls
            at full N=1024 (accumulating over head pairs), split into two
            drippable slices of 2 matmuls; cast + DMA ride on the second."""
            box = {}

            def emit(fc, lt=lt):
                if fc == 0:
                    box["psy"] = (pool or ps_proj).tile(
                        [128, 2, 512], F32, tag=tag, name=f"psy_{lt}"
                    )
                    box["y"] = y_pool.tile([128, 1024], BF16, tag="y", name=f"y_{lt}")
                psy, y_sb = box["psy"], box["y"]
                for p in range(NP):
                    nc.tensor.matmul(
                        psy[:, fc],
                        lhsT=outT_sb[:, p, lt * 128 : (lt + 1) * 128],
                        rhs=wo_sb[:, p, fc * 512 : (fc + 1) * 512],
                        start=(p == 0),
                        stop=(p == NP - 1),
                    )
                nc.vector.tensor_copy(y_sb[:, fc * 512 : (fc + 1) * 512], psy[:, fc])
                if fc == 1:
                    # gpsimd queue: keeps output DMAs off the input-DMA queue
                    nc.gpsimd.dma_start(y_d[lt * 128 : (lt + 1) * 128, :], y_sb[:])

            return [lambda fc=fc: emit(fc) for fc in range(2)]

        with tc.tile_pool(name="xw", bufs=1) as xw_pool:
            # DMA order = critical-path order: wqk + x8 (gate the first q/k
            # projections and scores), then bf16 x + wv (gate only the V
            # projection, which runs later).
            x8_sb = xw_pool.tile([128, CT, L], FP8)
            wqk_sb = xw_pool.tile([128, CT, 1024], FP8)
            xT_r = xT_d.rearrange("(t p) l -> p t l", p=128)
            x8_r = x8_d.rearrange("(t p) l -> p t l", p=128)
            wqk_r = wqk_d.rearrange("(t p) j -> p t j", p=128)
            for ct in range(CT):
                nc.sync.dma_start(wqk_sb[:, ct], wqk_r[:, ct])
                nc.sync.dma_start(x8_sb[:, ct], x8_r[:, ct])

            def V_pair(xbf_pool, wv_sb, g, proj_pool=False):
                """V projection for l-tiles 2g, 2g+1 as 4 drippable slices of
                4 matmuls. The bf16 x for these output rows arrives as a
                small [128, CT, 256] DMA slice fetched per pair (no resident
                32KB bf16-x tile)."""
                xv = xbf_pool.tile([128, CT, 256], BF16, tag="xv", name=f"xv_{g}")
                nc.sync.dma_start(xv[:], xT_r[:, :, g * 256 : (g + 1) * 256])
                slices = []
                for i in range(2):
                    lt = 2 * g + i
                    box = {}

                    def emit(half, i=i, lt=lt, xv=xv, box=box):
                        if half == 0:
                            if proj_pool:
                                pool, tag = ps_proj, "proj"
                            else:
                                pool, tag = (
                                    (ps_avA, "avA") if lt % 2 == 0 else (ps_avB, "avB")
                                )
                            box["psv"] = pool.tile(
                                [128, 512], F32, tag=tag, name=f"psv_{lt}"
                            )
                        psv = box["psv"]
                        for ct in range(4 * half, 4 * half + 4):
                            nc.tensor.matmul(
                                psv[:],
                                lhsT=xv[:, ct, i * 128 : (i + 1) * 128],
                                rhs=wv_sb[:, ct, :],
                                start=(ct == 0),
                                stop=(ct == CT - 1),
                            )
                        if half == 1:
                            nc.vector.tensor_copy(V_sb[:, lt, :], psv[:])

                    slices.append(lambda e=emit: e(0))
                    slices.append(lambda e=emit: e(1))
                return slices

            def qkT_unit_slices(jt, lh):
                """A qkT projection unit split into 4 drippable slices of
                2 DoubleRow matmuls (the psum group spans the slices)."""
                box = {}

                def emit(i, jt=jt, lh=lh):
                    if i == 0:
                        box["ps"] = ps_proj.tile(
                            [128, 2, 512], F32, tag="proj", name=f"proj_{jt}_{lh}"
                        )
                    ps = box["ps"]
                    for lc in range(2):
                        nc.tensor.matmul(
                            ps[:, lc],
                            lhsT=wqk_sb[
                                :, 2 * i : 2 * i + 2, jt * 128 : (jt + 1) * 128
                            ],
                            rhs=x8_sb[
                                :,
                                2 * i : 2 * i + 2,
                                lh * 1024 + lc * 512 : lh * 1024 + (lc + 1) * 512,
                            ],
                            start=(i == 0),
                            stop=(i == CP - 1),
                            perf_mode=DRMODE,
                        )
                    if i == 3:
                        nc.vector.tensor_scalar_add(
                            qkT_sb[:, jt, 2 * lh : 2 * lh + 2, :],
                            ps[:],
                            bqk_sb[:, jt : jt + 1],
                        )

                return [lambda i=i: emit(i) for i in range(4)]

            # Emission schedule: per chunk c we emit its AV groups (paced by
            # its exps), then the first 4 score k-tiles of chunk c+1 woven
            # between the two halves of c's row-sum pass (the sums can only
            # start once the AV groups close, i.e. after c's last exp), then
            # the normalize, then the remaining score k-tiles of c+1 with
            # projection work dripped one slice per k-tile.
            with tc.tile_pool(name="wv", bufs=1) as wv_pool, tc.tile_pool(
                name="xbf", bufs=2
            ) as xbf_pool:
                wv_sb = wv_pool.tile([128, CT, 512], BF16)
                nc.sync.dma_start(wv_sb[:], wv_d.rearrange("(t p) j -> p t j", p=128))

                # jq-major chunk order: chunk ci = (pair ci%4, jq ci//4).
                # Each q-block's outT completes after its 4th pair chunk, so
                # the y projection spreads over the whole kernel instead of
                # piling into the tail. Scores run two chunks ahead (3 exp
                # bufs); dripped qkT units are scheduled so each completes
                # before the woven scores that read it.
                chunks = [(i % NP, i // NP) for i in range(NP * JQ)]
                qkT_proj_unit(x8_sb, wqk_sb, 4, 0)
                qkT_proj_unit(x8_sb, wqk_sb, 0, 0)
                exp0 = exp_pool.tile([128, KT, 2, 512], BF16, tag="expT")
                for kt in range(8):
                    score_kt(0, 0, exp0, kt)
                qkT_proj_unit(x8_sb, wqk_sb, 4, 1)
                d = qkT_unit_slices(5, 0) + qkT_unit_slices(1, 0)
                for kt in range(8, KT):
                    d.pop(0)()
                    score_kt(0, 0, exp0, kt)
                # V slices drip into the ACT-paced score slots (the PE is
                # otherwise idle ~50% of each exp-paced score step)
                exp1 = exp_pool.tile([128, KT, 2, 512], BF16, tag="expT")
                d = qkT_unit_slices(5, 1) + V_pair(xbf_pool, wv_sb, 0)
                for kt in range(8):
                    d.pop(0)()
                    score_kt(1, 0, exp1, kt)
                d = qkT_unit_slices(6, 0) + qkT_unit_slices(2, 0)
                for kt in range(8, KT):
                    d.pop(0)()
                    score_kt(1, 0, exp1, kt)
                exp2 = exp_pool.tile([128, KT, 2, 512], BF16, tag="expT")
                d = qkT_unit_slices(6, 1) + V_pair(xbf_pool, wv_sb, 1)
                for kt in range(8):
                    d.pop(0)()
                    score_kt(2, 0, exp2, kt)
                d = qkT_unit_slices(7, 0) + qkT_unit_slices(3, 0)
                for kt in range(8, KT):
                    d.pop(0)()
                    score_kt(2, 0, exp2, kt)
                # early half of chunk (3,0)'s scores into the one-shot expX
                # tile (fed by the pair-3 q/k units just dripped above),
                # with the remaining prologue V pairs interleaved
                expX = expX_pool.tile([128, 4, 2, 512], BF16, tag="expX")
                d = V_pair(xbf_pool, wv_sb, 2) + V_pair(xbf_pool, wv_sb, 3)
                for kt in range(4):
                    d.pop(0)()
                    d.pop(0)()
                    score_kt(3, 0, expX, kt)

                exps = {0: exp0, 1: exp1, 2: exp2}
                # chunk (0,0) inline: V pairs 4-7 woven into its AV phase
                p, jq = chunks[0]
                expT = exps.pop(0)
                avA, avB = av_alloc()
                # ps_avA/avB hold this chunk's accumulators; route the V
                # psums through the proj pool instead
                vq = []
                for g in range(4, 8):
                    vq.extend(V_pair(xbf_pool, wv_sb, g, proj_pool=True))
                for kt in range(KT):
                    if vq:
                        vq.pop(0)()
                    av_mms(avA, avB, p, jq, expT, [kt])
                sum_mms(avA, avB, expT, range(0, KT))
                norm_part(p, jq, avA, avB)

            # wo loads after the prologue peak (first y unit at chunk 4),
            # reusing the SBUF freed by the wv/xbf pools
            wo_sb = wo_pool.tile([128, 4, F], BF16)
            nc.sync.dma_start(wo_sb[:], wo_d.rearrange("(t p) f -> p t f", p=128))

            # remaining qkT units (k1 for pair 3, then the lh=1 q units) and
            # the first 12 y units, dripped into specific chunks:
            #  - (7,1) at ci=1 completes before the (3,0) weave needs it (kt8)
            #  - q1 units (p,1) at ci 2-5, each well before the (p,2) weave
            #  - y l-tile lt drips once outT[:, :, jq(lt)] is complete, i.e.
            #    after chunk (3, jq) = ci 4*jq+3
            drip_sched = {
                2: [("qk", 0, 1)],
                3: [("qk", 1, 1)],
                4: [("qk", 2, 1), ("y", 0)],
                5: [("qk", 3, 1), ("y", 1)],
                6: [("y", 2)],
                7: [("y", 3)],
                8: [("y", 4)],
                9: [("y", 5)],
                10: [("y", 6)],
                11: [("y", 7)],
                12: [("y", 8), ("y", 10)],
                13: [("y", 9), ("y", 11)],
            }
            for ci in range(1, len(chunks)):
                p, jq = chunks[ci]
                nxt = chunks[ci + 2] if ci + 2 < len(chunks) else None
                emit_nxt = nxt is not None and (ci + 2) not in exps
                if emit_nxt:
                    exps[ci + 2] = exp_pool.tile([128, KT, 2, 512], BF16, tag="expT", name=f"expT_{ci+2}")
                expT = exps.pop(ci)
                avA, avB = av_alloc()
                # chunk 3's kt0-7 scores already ran in the prologue (expX),
                # so ci=1 only weaves its kt8-15
                wq = list(range(4, KT)) if ci == 1 else list(range(KT))

                def w():
                    if emit_nxt and wq:
                        score_kt(*nxt, exps[ci + 2], wq.pop(0))

                # the pair-3 k1 unit rides the ci=1 av phase (complete
                # before the first kt8 weave slot needs it)
                avdrip = qkT_unit_slices(7, 1) if ci == 1 else []
                for kt in range(KT - 1):
                    if avdrip:
                        avdrip.pop(0)()
                    if ci == 3:
                        av_mms(avA, avB, p, jq, expX if kt < 4 else expT, [kt])
                    else:
                        av_mms(avA, avB, p, jq, expT, [kt])
                    if kt == 9:
                        # mid-av weave slot: the S pool has drained the
                        # previous chunk's last exps by now, so this feeds
                        # ACT ~2us earlier at each chunk boundary without
                        # risking a stall at the head of the PE queue
                        w()
                w()
                av_mms(avA, avB, p, jq, expT, [KT - 1])
                w()
                if ci == 3:
                    sum_mms(avA, avB, expX, range(0, 4))
                    sum_mms(avA, avB, expT, range(4, 6))
                else:
                    sum_mms(avA, avB, expT, range(0, 6))
                w()
                sum_mms(avA, avB, expT, range(6, 11))
                w()
                sum_mms(avA, avB, expT, range(11, KT))
                w()
                norm_part(p, jq, avA, avB)

                drip = []
                for job in drip_sched.get(ci, []):
                    if job[0] == "qk":
                        drip.extend(qkT_unit_slices(job[1], job[2]))
                    else:
                        drip.extend(y_unit_slices(job[1]))
                for _ in range(5, KT):
                    if drip:
                        drip.pop(0)()
                    w()
                while drip:
                    drip.pop(0)()
                while emit_nxt and wq:
                    w()

            # tail: the last q-block's y units, double-buffered through the
            # now-idle score psum pool
            for lt in range(12, 16):
                for s in y_unit_slices(lt, pool=ps_s, tag="s"):
                    s()

            if DEBUG:
                nc.gpsimd.dma_start(dbg_qkT_d[:], qkT_sb[:])
                nc.gpsimd.dma_start(dbg_V_d[:], V_sb[:])
                nc.gpsimd.dma_start(dbg_outT_d[:], outT_sb[:])

    nc.finalize()
    return nc


_NC_CACHE = None


def _get_program():
    global _NC_CACHE
    if _NC_CACHE is None:
        _NC_CACHE = _build_program()
    return _NC_CACHE


def _make_in_maps(x, W_in, b_in, W_out):
    f8 = ml_dtypes.float8_e4m3
    bf = ml_dtypes.bfloat16
    in_maps = []
    for c in range(NCORES):
        n, g = c // 2, c % 2
        h0 = g * HG  # first global head
        j0 = h0 * D  # 512*g
        xT = np.ascontiguousarray(x[n].T).astype(bf)  # [C, L]
        x8 = xT.astype(f8)
        wqk = (
            WS
            * np.concatenate(
                [W_in[:, j0 : j0 + 512], W_in[:, QKV + j0 : QKV + j0 + 512]], axis=1
            )
        ).astype(f8)
        wv = np.ascontiguousarray(W_in[:, 2 * QKV + j0 : 2 * QKV + j0 + 512]).astype(bf)
        wo = np.ascontiguousarray(W_out[j0 : j0 + 512, :]).astype(bf)
        bqk = (
            (WS * np.concatenate([b_in[j0 : j0 + 512], b_in[QKV + j0 : QKV + j0 + 512]]))
            .astype(np.float32)
            .reshape(8, 128)
            .T.copy()
        )
        in_maps.append(
            {"xT": xT, "x8": x8, "wqk": wqk, "wv": wv, "wo": wo, "bqk": bqk}
        )
    return in_maps


def kernel(x, W_in, b_in, W_out, b_out):
    global LAST_RESULTS
    x = np.asarray(x, dtype=np.float32)
    W_in = np.asarray(W_in, dtype=np.float32)
    b_in = np.asarray(b_in, dtype=np.float32)
    W_out = np.asarray(W_out, dtype=np.float32)
    b_out = np.asarray(b_out, dtype=np.float32)

    nc = _get_program()
    in_maps = _make_in_maps(x, W_in, b_in, W_out)
    res = run_bass_kernel_spmd(nc, in_maps, list(range(NCORES)), trace=TRACE)
    LAST_RESULTS = res

    # host bias: b_out + b_v @ W_out  (b_v enters linearly through the
    # softmax-normalized value average: A@(V+b_v) = A@V + b_v)
    host_bias = (
        b_out.astype(np.float64)
        + b_in[2 * QKV :].astype(np.float64) @ W_out.astype(np.float64)
    ).astype(np.float32)

    out = np.empty((N, L, F), dtype=np.float32)
    for n in range(N):
        y0 = np.asarray(res.results[2 * n]["y"], dtype=np.float32)
        y1 = np.asarray(res.results[2 * n + 1]["y"], dtype=np.float32)
        out[n] = y0 + y1 + host_bias
    return out
